# revision 12
# baseline (speedup 1.0000x reference)
"""Trainium2 Bass kernel for a 2-layer GRU (B=256, T=256, D=128, H=512) + FC head.

Strategy: data-parallel over batch (B=32 per core, 8 cores), single SPMD launch.
Everything stays on-chip after the initial weight/x loads.

Layouts (per core, local batch 32):
  - All recurrent tensors live "transposed": gh.T = [gate_dim on partitions, batch free].
    gh.T M-tiles (12 of [128, 32]) -> r = tiles 0-3, z = 4-7, n = 8-11.
    h.T stored as [128, 4, 32]: h-slab k (h dims 128k..128k+127) at [:, k, :].
    h_new.T comes out in exactly the rhs layout the next step's matmuls need
    (contraction over h on partitions) -> zero transposes in the whole kernel.
  - x passed from host pre-transposed: xT [128(D), T, 32(batch)].
  - Weights pre-transposed on host (W.T K-tiles).
  - L0 input projection folded into each step's PSUM accumulation group (K=D=128).
  - L1 input projection (K=512) computed in CH-step chunks from the y0 ring.
  - Biases: precomputed broadcast tiles added with one tensor_tensor each.
"""

import os
import sys

sys.path.insert(0, "/opt/trn_rl_repo")

import ml_dtypes
import numpy as np

import concourse.bass as bass
import concourse.tile as tile
from concourse import mybir
from concourse.bass_utils import run_bass_kernel_spmd

AF = mybir.ActivationFunctionType
ALU = mybir.AluOpType
F32 = mybir.dt.float32

B, T, D, H, C = 256, 256, 128, 512, 10
NCORES = 8
BL = B // NCORES          # 32 batch per core
G3 = 3 * H                # 1536
NK = H // 128             # 4 h k-tiles
NM = G3 // 128            # 12 gate m-tiles
RING = 8                  # y0 ring slots (steps)
CH = 4                    # xg1 chunk size (steps)

_CACHE = {}


def _split_multiwaits(nc):
    """Walrus/HW allow a single sync-wait per engine instruction. Tile can
    emit several; hoist extras into same-engine NoOps placed just before."""
    import json as _json
    import types as _types

    d = _json.loads(mybir.module_to_json_bytes(nc.m))
    nsw = 0
    for fn in d["functions"]:
        for blk in fn["blocks"]:
            out = []
            for ins in blk["instructions"]:
                si = ins.get("sync_info") or {}
                ow = si.get("on_wait") or []
                if len(ow) > 1:
                    for w in ow[:-1]:
                        out.append({
                            "engine": ins["engine"],
                            "ins": [],
                            "outs": [],
                            "name": f"I-SW-{nsw}",
                            "opcode": "NoOp",
                            "sync_info": {"on_update": [], "on_wait": [w]},
                        })
                        nsw += 1
                    si["on_wait"] = [ow[-1]]
                out.append(ins)
            blk["instructions"] = out
    blob = _json.dumps(d).encode()
    nc.to_json_bytes = _types.MethodType(lambda self: blob, nc)
    return nsw


def _build(dt_w, n_steps):
    """Build the Bass program. dt_w: weight/x/h/gate dtype. Returns nc."""
    DT = dt_w
    nc = bass.Bass("TRN2", target_bir_lowering=False, debug=False, num_devices=NCORES)

    nwf = 8 * BL + 3 * NK * BL + NM * CH * BL + 2
    fmul = 4 // mybir.dt.size(DT)          # DT cols per f32 value
    nwd = n_steps * BL + G3 + 3 * NK * G3 + NK * 128 + C + 2 + nwf * fmul
    d_wb = nc.dram_tensor("wb", [128, nwd], DT, kind="ExternalInput").ap()
    d_out = nc.dram_tensor("out", [C, BL], F32, kind="ExternalOutput").ap()

    with tile.TileContext(nc) as tc:
        with (
            tc.tile_pool(name="w", bufs=1) as wp,
            tc.tile_pool(name="ring", bufs=1) as ringp,
            tc.tile_pool(name="xg1", bufs=2) as xg1p,
            tc.tile_pool(name="h1", bufs=3) as h1p,
            tc.tile_pool(name="g", bufs=3) as gp,
            tc.tile_pool(name="g2", bufs=3) as gp2,
            tc.tile_pool(name="p0rz", bufs=1, space="PSUM") as p0rzp,
            tc.tile_pool(name="p0hn", bufs=1, space="PSUM") as p0hnp,
            tc.tile_pool(name="p0xn", bufs=1, space="PSUM") as p0xnp,
            tc.tile_pool(name="p1rz", bufs=1, space="PSUM") as p1rzp,
            tc.tile_pool(name="p1hn", bufs=1, space="PSUM") as p1hnp,
            tc.tile_pool(name="pxg1", bufs=1, space="PSUM") as pxg1p,
        ):
            # ---- one persistent SBUF blob, ONE load DMA (HW allows only a
            # single sync-wait per instruction, so all consumers join on it) ----
            wb = wp.tile([128, nwd], DT)
            y0r = ringp.tile([128, RING, NK, BL], DT)   # y0 / h0 ring
            h1init = ringp.tile([128, NK, BL], DT)
            nc.sync.dma_start(wb[:], d_wb[:])

            # blob views
            o = 0
            def take(n):
                nonlocal o
                a, o = o, o + n
                return a
            o_xT = take(n_steps * BL)
            o_wih0 = take(G3)
            o_whh0 = take(NK * G3)
            o_wih1 = take(NK * G3)
            o_whh1 = take(NK * G3)
            o_fc1w = take(NK * 128)
            o_fc2w = take(C + 2)
            o_f32 = take(nwf * fmul)
            xT = wb[:, o_xT:o_xT + n_steps * BL].rearrange("p (t b) -> p t b", b=BL)
            fc2w = wb[:, o_fc2w:o_fc2w + C]
            fbt = wb[:, o_f32:o_f32 + nwf * fmul].bitcast(F32)
            o = 0
            o_brz0 = take(8 * BL)
            o_bxn0 = take(NK * BL)
            o_bhn0 = take(NK * BL)
            o_bhn1 = take(NK * BL)
            o_bx1 = take(NM * CH * BL)
            o_fc1b = take(1)
            o_fc2b = take(1)
            brz0 = fbt[:, o_brz0:o_brz0 + 8 * BL].rearrange("p (m b) -> p m b", b=BL)
            bxn0 = fbt[:, o_bxn0:o_bxn0 + NK * BL].rearrange("p (m b) -> p m b", b=BL)
            bhn0 = fbt[:, o_bhn0:o_bhn0 + NK * BL].rearrange("p (m b) -> p m b", b=BL)
            bhn1 = fbt[:, o_bhn1:o_bhn1 + NK * BL].rearrange("p (m b) -> p m b", b=BL)
            bx1 = fbt[:, o_bx1:o_bx1 + NM * CH * BL].rearrange("p (m s b) -> p m s b", s=CH, b=BL)
            fc1b = fbt[:, o_fc1b:o_fc1b + 1]
            fc2b = fbt[:, o_fc2b:o_fc2b + 1]

            def wih0_t(m):
                return wb[:, o_wih0 + m * 128:o_wih0 + (m + 1) * 128]
            def whh0_t(k, m):
                return wb[:, o_whh0 + k * G3 + m * 128:o_whh0 + k * G3 + (m + 1) * 128]
            def wih1_t(k, m):
                return wb[:, o_wih1 + k * G3 + m * 128:o_wih1 + k * G3 + (m + 1) * 128]
            def whh1_t(k, m):
                return wb[:, o_whh1 + k * G3 + m * 128:o_whh1 + k * G3 + (m + 1) * 128]
            def fc1w_t(k):
                return wb[:, o_fc1w + k * 128:o_fc1w + (k + 1) * 128]

            # HW allows only ONE sync-wait per instruction. Prime DVE and ACT
            # with a tiny read of the blob so their clocks observe the load-DMA
            # sem once; afterwards every instruction needs at most one wait.
            prdve = gp.tile([1, 4], DT, tag="prime")
            nc.vector.tensor_copy(prdve[:], wb[0:1, 0:4])
            pract = gp.tile([1, 4], DT, tag="prime2")
            nc.scalar.copy(pract[:], wb[0:1, 0:4])
            nc.vector.memset(y0r[:, RING - 1], 0.0)  # h0(t=-1) = 0
            nc.vector.memset(h1init[:], 0.0)

            def l0_step(t):
                rs = (t + RING - 1) % RING   # h_old ring slot
                ws = t % RING                # h_new ring slot
                prz = p0rzp.tile([128, 8, BL], F32)
                phn = p0hnp.tile([128, NK, BL], F32)
                pxn = p0xnp.tile([128, NK, BL], F32)
                for m in range(8):           # r, z
                    o = prz[:, m]
                    nc.tensor.matmul(o, wih0_t(m),
                                     xT[:, t], start=True, stop=False)
                    for k in range(NK):
                        nc.tensor.matmul(
                            o, whh0_t(k, m),
                            y0r[:, rs, k], start=False, stop=(k == NK - 1))
                for m in range(8, NM):       # n: keep x-part and h-part separate
                    j = m - 8
                    nc.tensor.matmul(pxn[:, j], wih0_t(m),
                                     xT[:, t], start=True, stop=True)
                    for k in range(NK):
                        nc.tensor.matmul(
                            phn[:, j], whh0_t(k, m),
                            y0r[:, rs, k], start=(k == 0), stop=(k == NK - 1))
                t0 = gp.tile([128, 8, BL], DT, tag="t0")
                nc.vector.tensor_add(t0[:], prz[:], brz0[:])
                sig = gp.tile([128, 8, BL], DT, tag="sig")
                nc.scalar.activation(sig[:], t0[:], AF.Sigmoid)
                hnp = gp.tile([128, NK, BL], DT, tag="hnp")
                nc.vector.tensor_add(hnp[:], phn[:], bhn0[:])
                xnp = gp.tile([128, NK, BL], DT, tag="xnp")
                nc.vector.tensor_add(xnp[:], pxn[:], bxn0[:])
                t2 = gp.tile([128, NK, BL], DT, tag="t2")
                nc.vector.tensor_mul(t2[:], sig[:, 0:4], hnp[:])
                t3 = gp.tile([128, NK, BL], DT, tag="t3")
                nc.vector.tensor_add(t3[:], t2[:], xnp[:])
                ntl = gp.tile([128, NK, BL], DT, tag="ntl")
                nc.scalar.activation(ntl[:], t3[:], AF.Tanh)
                v = gp.tile([128, NK, BL], DT, tag="v")
                nc.vector.tensor_scalar(v[:], sig[:, 4:8], -1.0, 1.0,
                                        ALU.mult, ALU.add)
                u = gp.tile([128, NK, BL], DT, tag="u")
                nc.vector.tensor_mul(u[:], sig[:, 4:8], y0r[:, rs])
                m1 = gp.tile([128, NK, BL], DT, tag="m1")
                nc.vector.tensor_mul(m1[:], v[:], ntl[:])
                nc.vector.tensor_add(y0r[:, ws], m1[:], u[:])

            def xg1_gemm(c):
                """project y0 steps [CH*c, CH*c+CH) -> PSUM chunk."""
                r0 = (CH * c) % RING
                pxg1 = pxg1p.tile([128, NM, CH, BL], F32)
                for m in range(NM):
                    for k in range(NK):
                        nc.tensor.matmul(
                            pxg1[:, m], wih1_t(k, m),
                            y0r[:, r0:r0 + CH, k], start=(k == 0),
                            stop=(k == NK - 1))
                xg1 = xg1p.tile([128, NM, CH, BL], F32, tag="xg1")
                return pxg1, xg1

            def xg1_slice(pxg1, xg1, s):
                """one step-slice of the chunk bias add (spreads the DVE cost
                so no single 1.8us op blocks a chain op behind it)."""
                nc.vector.tensor_add(xg1[:, :, s], pxg1[:, :, s], bx1[:, :, s])

            def l1_step(tau, xg1, h_old):
                s = tau % CH
                prz = p1rzp.tile([128, 8, BL], F32)
                phn = p1hnp.tile([128, NK, BL], F32)
                for m in range(8):
                    for k in range(NK):
                        nc.tensor.matmul(
                            prz[:, m], whh1_t(k, m),
                            h_old[:, k], start=(k == 0), stop=(k == NK - 1))
                for m in range(8, NM):
                    for k in range(NK):
                        nc.tensor.matmul(
                            phn[:, m - 8], whh1_t(k, m),
                            h_old[:, k], start=(k == 0), stop=(k == NK - 1))
                t0 = gp2.tile([128, 8, BL], DT, tag="t0b")
                nc.vector.tensor_add(t0[:], prz[:], xg1[:, 0:8, s])
                sig = gp2.tile([128, 8, BL], DT, tag="sigb")
                nc.scalar.activation(sig[:], t0[:], AF.Sigmoid)
                hnp = gp2.tile([128, NK, BL], DT, tag="hnpb")
                nc.vector.tensor_add(hnp[:], phn[:], bhn1[:])
                t2 = gp2.tile([128, NK, BL], F32, tag="t2b")
                nc.vector.tensor_mul(t2[:], sig[:, 0:4], hnp[:])
                t3 = gp2.tile([128, NK, BL], DT, tag="t3b")
                nc.vector.tensor_add(t3[:], t2[:], xg1[:, 8:NM, s])
                ntl = gp2.tile([128, NK, BL], DT, tag="ntlb")
                nc.scalar.activation(ntl[:], t3[:], AF.Tanh)
                v = gp2.tile([128, NK, BL], DT, tag="vb")
                nc.vector.tensor_scalar(v[:], sig[:, 4:8], -1.0, 1.0,
                                        ALU.mult, ALU.add)
                u = gp2.tile([128, NK, BL], DT, tag="ub")
                nc.vector.tensor_mul(u[:], sig[:, 4:8], h_old[:])
                m1 = gp2.tile([128, NK, BL], DT, tag="m1b")
                nc.vector.tensor_mul(m1[:], v[:], ntl[:])
                h_new = h1p.tile([128, NK, BL], DT, tag="h1")
                nc.vector.tensor_add(h_new[:], m1[:], u[:])
                return h_new

            # ---- main pipeline: L1 lags L0 by CH steps ----
            # tile_wait_until staircase = sim-time floors that give the Tile
            # scheduler the real step cadence, so each engine's stream is
            # ordered [L0 chain of t] -> [L1 chain of tau] -> [chunk tail]
            # instead of the cost-model's optimistic interleave (which put
            # L1's PSUM add ahead of L0's ready chain ops on DVE).
            PST = 0.006  # ms per step
            h1 = h1init
            fill_p = fill_x = cons = None
            fslice = CH
            for t in range(n_steps):
                with tc.tile_wait_until(t * PST):
                    l0_step(t)
                if t % CH == CH - 1:
                    with tc.tile_wait_until((t + 0.6) * PST):
                        fill_p, fill_x = xg1_gemm(t // CH)
                    fslice = 0
                tau = t - CH
                if tau >= 0:
                    if tau % CH == 0:
                        cons = fill_x
                    with tc.tile_wait_until((t + 0.55) * PST):
                        h1 = l1_step(tau, cons, h1)
                if fslice < CH:
                    with tc.tile_wait_until((t + 0.85) * PST):
                        xg1_slice(fill_p, fill_x, fslice)
                    fslice += 1
            for j, tau in enumerate(range(n_steps - CH, n_steps)):
                t = n_steps + j
                if tau % CH == 0:
                    cons = fill_x
                if fslice < CH:
                    with tc.tile_wait_until((t + 0.2) * PST):
                        xg1_slice(fill_p, fill_x, fslice)
                    fslice += 1
                with tc.tile_wait_until((t + 0.55) * PST):
                    h1 = l1_step(tau, cons, h1)

            # ---- FC head ----
            hr = gp.tile([128, NK, BL], DT, tag="hr")
            nc.scalar.activation(hr[:], h1[:], AF.Relu)
            pfc = p0rzp.tile([128, BL], F32, tag="prz")
            for k in range(NK):
                nc.tensor.matmul(pfc[:], fc1w_t(k), hr[:, k],
                                 start=(k == 0), stop=(k == NK - 1))
            o1 = gp.tile([128, BL], DT, tag="o1")
            nc.scalar.activation(o1[:], pfc[:], AF.Relu, bias=fc1b[:])
            pfc2 = p0hnp.tile([C, BL], F32, tag="phn")
            nc.tensor.matmul(pfc2[:], fc2w[:], o1[:], start=True, stop=True)
            ofin = gp.tile([C, BL], F32, tag="ofin")
            nc.scalar.activation(ofin[:], pfc2[:], AF.Identity, bias=fc2b[0:C, :])
            nc.sync.dma_start(d_out[:], ofin[:])

    return nc


def _prep_inputs(inputs, dt_np, n_steps):
    """Host-side layout prep: pack per-core DT blob + shared F32 blob."""
    f32 = np.float32
    x = inputs["x"][:, :n_steps, :]
    b_ih0, b_hh0 = inputs["b_ih0"].astype(f32), inputs["b_hh0"].astype(f32)
    b_ih1, b_hh1 = inputs["b_ih1"].astype(f32), inputs["b_hh1"].astype(f32)

    def kt(W):  # [3H, K] -> [128, NKw*3H] K-tiles of W.T side by side
        Wt = W.T.astype(dt_np)  # [K, 3H]
        nk = W.shape[1] // 128
        return Wt.reshape(nk, 128, G3).transpose(1, 0, 2).reshape(128, nk * G3)

    wih0 = inputs["W_ih0"].T.astype(dt_np)               # [128, 1536]
    whh0, wih1, whh1 = kt(inputs["W_hh0"]), kt(inputs["W_ih1"]), kt(inputs["W_hh1"])
    fc1w = (inputs["fc1_w"].T.astype(dt_np)              # [512, 128] -> [128, 4*128]
            .reshape(NK, 128, 128).transpose(1, 0, 2).reshape(128, NK * 128))
    fc2w = inputs["fc2_w"].T.astype(dt_np)               # [128, 10]

    def bcast(bias, nm, reps):  # [nm*128] -> [128, nm*reps]
        return np.broadcast_to(
            bias.reshape(nm, 128).T[:, :, None], (128, nm, reps)
        ).reshape(128, nm * reps)

    brz0 = bcast((b_ih0 + b_hh0)[:1024], 8, BL)
    bxn0 = bcast(b_ih0[1024:], NK, BL)
    bhn0 = bcast(b_hh0[1024:], NK, BL)
    bhn1 = bcast(b_hh1[1024:], NK, BL)
    bfull = b_ih1.copy()
    bfull[:1024] += b_hh1[:1024]
    bx1 = bcast(bfull, NM, CH * BL)
    fc1b = inputs["fc1_b"].astype(f32).reshape(128, 1)
    fc2b = np.zeros((128, 1), f32)
    fc2b[:C, 0] = inputs["fc2_b"].astype(f32)

    fb = np.ascontiguousarray(np.concatenate(
        [brz0, bxn0, bhn0, bhn1, bx1, fc1b, fc2b], axis=1)).astype(f32)
    fb_dt = fb.view(np.uint8).reshape(128, -1).view(dt_np)  # raw bytes as DT cols
    pad = np.zeros((128, 2), dt_np)

    wtail = np.concatenate(
        [wih0, whh0, wih1, whh1, fc1w, fc2w, pad, fb_dt], axis=1)
    in_maps = []
    for c in range(NCORES):
        xc = x[c * BL:(c + 1) * BL]                      # [32, T, 128]
        xTc = xc.transpose(2, 1, 0).reshape(128, n_steps * BL).astype(dt_np)
        wbc = np.ascontiguousarray(np.concatenate([xTc, wtail], axis=1))
        in_maps.append(dict(wb=wbc))
    return in_maps


def run(inputs, dtype="float32", n_steps=T, trace=False):
    dt_w = F32 if dtype == "float32" else mybir.dt.bfloat16
    dt_np = np.float32 if dtype == "float32" else ml_dtypes.bfloat16
    key = (dtype, n_steps)
    if key not in _CACHE:
        nc = _build(dt_w, n_steps)
        n = _split_multiwaits(nc)
        print(f"split {n} multi-waits", flush=True)
        _CACHE[key] = nc
    nc = _CACHE[key]
    in_maps = _prep_inputs(inputs, dt_np, n_steps)
    res = run_bass_kernel_spmd(nc, in_maps, list(range(NCORES)), trace=trace)
    outs = [r["out"] for r in res.results]  # each [C, BL]
    full = np.concatenate([o.T for o in outs], axis=0).astype(np.float32)
    return full, res


def kernel(**inputs):
    full, _ = run(inputs, dtype=os.environ.get("GRU_DTYPE", "bfloat16"))
    return full

